# revision 23
# baseline (speedup 1.0000x reference)
"""Trainium2 Bass kernel for DigitConvolutionalModel.

Math: the 3x3 valid conv is a linear map, so it folds into the first Linear
layer on the host (O(1) w.r.t. batch):  out = relu(x @ W_eff + b1) @ w2.T + b2
with W_eff[784, 128].  Distribution is pure data parallel: batch sharded
across 8 NeuronCores, weights replicated, each core computing [10, 8192].

dtypes: x ships as fp8 e3m4 (4 mantissa bits; subnormals handled exactly by
the PE's FP22 upconvert) against fp16 weights — this halves HBM traffic vs
fp16 and costs ~1.3e-2 max rel error on this data (gate: 2e-2).  fp32 PSUM
accumulation; h is emitted fp16 for the second matmul; logits are stored
fp16 (4.9e-4 relative — noise here) and upconverted on host.

PE-array packing: batch tiles process in GROUPS of [2,4,4,4,2].  Within a
group, tile j's K=16 remainder matmul runs in PE row-strip 32j via
tile_position=(32j, 0), and its M=10 second matmul runs in column strip
32j via tile_position=(0, 32j) — row/col-disjoint matmuls execute
concurrently, so a group's worth of remainder (or mm2) passes costs ~1
pass.  The 2-tile first group hands the DMA generator burst-slack right
where tile 2's delivery is tightest; the 2-tile last group halves the
end-of-kernel relu/mm2/store tail.

DMA plan: the two HWDGE rings (sync/scalar) share one descriptor
generator at ~20ns/descriptor; every SBUF-bound DMA costs one descriptor
per partition and lines cap at 8KB, so 2-tile x groups (6144B lines,
~305GB/s) are optimal — just above the PE's 296GB/s consumption.  The
generator serves commands in global trigger order (the rings' queues
interleave position by position), so the queue split below yields exactly
consumption order: wm, p01, p23, ..., p1415.  wm must lead a HWDGE ring:
the gpsimd SWDGE queue (which carries all small tensors and the
non-final output stores) only starts generating ~11.8us into the kernel.

The group epilogue pipeline: group g's mm2 burst is emitted after group
g+1's second mm1 chain, so the PE never waits on the relu chain and the
bias-adds clear the vector/scalar queues before the next relu wave.
Relus split vector/scalar per group; b2 is replicated per row-strip so
strip-sliced bias-adds work for any group size; each group stores its
strip block in one SWDGE DMA (final group: ring halves fired per
bias-add half).
"""

import numpy as np
import ml_dtypes

import concourse.bass as bass  # noqa: F401  (bass registers mybir lowerings)
import concourse.mybir as mybir
import concourse.tile as tile
from concourse import bacc
from concourse.bass_utils import run_bass_kernel_spmd

N_CORES = 8
B = 65536
B_SH = B // N_CORES  # 8192 rows per core
D = 784              # 28*28 input features
DM = 768             # features in the main 128-partition stream
DR = D - DM          # 16 remainder features
H = 128              # hidden
OUT = 10
KT = 128             # contraction tile = full partition dim
NK = DM // KT        # 6 main K-tiles
NB = 512             # batch columns per tile (= one fp32 PSUM bank)
NT = B_SH // NB      # 16 batch tiles

GROUP_SIZES = [2, 4, 4, 4, 2]
NG = len(GROUP_SIZES)
GROUP_START = [sum(GROUP_SIZES[:i]) for i in range(NG)]

N_WARM = 14  # bridge PE from engine-start (~7.5us) to wm+pair(0,1)
             # arrival (~12.5-13.4us) with no idle gap, so the HAM clock
             # never re-throttles (a >1us PE gap can halve the clock
             # for ~6us)

_CACHE = {}


def _build_nc():
    f32 = mybir.dt.float32
    f16 = mybir.dt.float16
    f8 = mybir.dt.float8e3
    nc = bacc.Bacc("TRN2", target_bir_lowering=False, debug=False,
                   num_devices=N_CORES)
    # x is flat per-partition so DMA commands can be exactly 8KB lines
    # (the descriptor max): byte s*NB..s*NB+NB of partition k is batch
    # tile s//NK, k-slice s%NK
    XBYTES = NT * NK * NB
    xtp = nc.dram_tensor("xtp", [KT, XBYTES], f8,
                         kind="ExternalInput").ap()
    # remainder features per row-strip: [32j+r, g, c] = feature 768+r of
    # group g's j-th tile (r<16; rows 16..31 of each strip are zero pad)
    xr4 = nc.dram_tensor("xr4", [KT, NG, NB], f8, kind="ExternalInput").ap()
    wm = nc.dram_tensor("wm", [KT, NK, H], f16, kind="ExternalInput").ap()
    # remainder weights replicated into each row-strip
    wr4 = nc.dram_tensor("wr4", [KT, H], f16, kind="ExternalInput").ap()
    w2t = nc.dram_tensor("w2t", [H, OUT], f16, kind="ExternalInput").ap()
    # biasd[:, 0] = b1; biasd[32j+r, 1] = b2[r] (r<10)
    biasd = nc.dram_tensor("biasd", [KT, 2], f32, kind="ExternalInput").ap()
    # out4[32j+r, g, c] = logit r of batch row (GROUP_START[g]+j)*512+c
    # (r<10; rows 10..31 of each strip, and strips >= group size, are
    # don't-care)
    out4 = nc.dram_tensor("out4", [KT, NG, NB], f16,
                          kind="ExternalOutput").ap()

    relu = mybir.ActivationFunctionType.Relu
    ident = mybir.ActivationFunctionType.Identity

    with tile.TileContext(nc) as tc:
        with (
            tc.tile_pool(name="wpool", bufs=1) as wpool,
            tc.tile_pool(name="xpool", bufs=1) as xpool,
            tc.tile_pool(name="hpool", bufs=8) as hpool,
            tc.tile_pool(name="opool", bufs=2) as opool,
            tc.tile_pool(name="ps1", bufs=6, space="PSUM") as ps1pool,
            tc.tile_pool(name="ps2", bufs=2, space="PSUM") as ps2pool,
        ):
            x_sb = xpool.tile([KT, XBYTES], f8)
            w_sb = wpool.tile([KT, NK, H], f16)
            w2_sb = wpool.tile([H, OUT], f16)
            bias_sb = wpool.tile([KT, 2], f32)
            wr_sb = wpool.tile([KT, H], f16)
            xr_sb = wpool.tile([KT, NG, NB], f8)

            # x streams in six exact-8KB-line commands (128 descriptors
            # each, the hardware line max — 33% denser than 2-tile 6KB
            # pairs).  Queue split chosen so global trigger order =
            # consumption order: wm, c0, c1, ..., c5.
            CH = XBYTES // 6  # 8192
            nc.sync.dma_start(w_sb[:], wm[:])
            for ci in range(6):
                eng = (nc.scalar, nc.sync)[ci % 2]
                eng.dma_start(x_sb[:, ci * CH:(ci + 1) * CH],
                              xtp[:, ci * CH:(ci + 1) * CH])
            # SWDGE: remainder stream first (first rem burst needs
            # wr4/xr4 by ~15.5us), then bias (~16us), then w2t (~19us)
            nc.gpsimd.dma_start(wr_sb[:], wr4[:])
            nc.gpsimd.dma_start(xr_sb[:], xr4[:])
            nc.gpsimd.dma_start(bias_sb[:], biasd[:])
            nc.gpsimd.dma_start(w2_sb[:], w2t[:])

            warm_x = wpool.tile([KT, NB], f16)
            nc.vector.memset(warm_x[:], 0.0)
            warm_ps = ps1pool.tile([H, NB], f32, tag="ps1")
            for _ in range(N_WARM):
                nc.tensor.matmul(warm_ps[:], lhsT=warm_x[:, 0:H],
                                 rhs=warm_x[:], start=True, stop=True)

            def v_relu(h_sb, ps1):
                nc.vector.tensor_scalar(
                    h_sb[:], ps1[:], bias_sb[:, 0:1], 0.0,
                    mybir.AluOpType.add, mybir.AluOpType.max)

            def s_relu(h_sb, ps1):
                nc.scalar.activation(h_sb[:], ps1[:], relu,
                                     bias=bias_sb[:, 0:1])

            def rem_relu(g, ps1s, relu_fns=None):
                # remainder burst: row-tiled K=16 passes, one per strip,
                # then the relu wave split across DVE and Act so the
                # chain is <=2 deep per engine; emission order lets both
                # engines start immediately
                m = len(ps1s)
                for j in range(m):
                    nc.tensor.matmul(
                        ps1s[j][:],
                        lhsT=wr_sb[32 * j:32 * j + DR, :],
                        rhs=xr_sb[32 * j:32 * j + DR, g, :],
                        start=False, stop=True,
                        tile_position=(32 * j, 0),
                    )
                if relu_fns is None:
                    order = (0, 2, 1, 3) if m == 4 else (0, 1)
                    relu_fns = {j: (v_relu if j < m // 2 else s_relu)
                                for j in order}
                hs = [None] * m
                for j, fn in relu_fns.items():
                    h_sb = hpool.tile([H, NB], f16, name="h_sb")
                    fn(h_sb, ps1s[j])
                    hs[j] = h_sb
                return hs

            def mm2_store_burst(g, hs, o_mode="split"):
                # col-tiled mm2 passes into one shared PSUM bank; each
                # pass fires as its h lands (independent col strips)
                m = len(hs)
                ps2 = ps2pool.tile([KT, NB], f32, name="ps2")
                for j in range(m):
                    nc.tensor.matmul(
                        ps2[32 * j:32 * j + OUT, :],
                        lhsT=w2_sb[:], rhs=hs[j][:],
                        start=True, stop=True,
                        tile_position=(0, 32 * j),
                    )
                rows = 32 * m
                o_sb = opool.tile([KT, NB], f16, name="o_sb")
                if o_mode == "vector_final":
                    # final group: vector-only bias-add (the scalar
                    # queue shows ~1us of semaphore lag on trailing o
                    # ops), store halves on both now-idle HWDGE rings
                    nc.vector.tensor_scalar_add(
                        o_sb[0:rows, :], ps2[0:rows, :], bias_sb[0:rows, 1:2])
                    half = rows // 2
                    nc.sync.dma_start(out4[0:half, g, :], o_sb[0:half, :])
                    nc.scalar.dma_start(out4[half:rows, g, :],
                                        o_sb[half:rows, :])
                    return
                if o_mode == "scalar_full":
                    # penultimate group: keep the whole bias-add off the
                    # vector queue so the final relu wave owns it
                    nc.scalar.activation(
                        o_sb[0:rows, :], ps2[0:rows, :], ident,
                        bias=bias_sb[0:rows, 1:2])
                else:
                    half = rows // 2
                    nc.vector.tensor_scalar_add(
                        o_sb[0:half, :], ps2[0:half, :], bias_sb[0:half, 1:2])
                    nc.scalar.activation(
                        o_sb[half:rows, :], ps2[half:rows, :], ident,
                        bias=bias_sb[half:rows, 1:2])
                nc.gpsimd.dma_start(out4[0:rows, g, :], o_sb[0:rows, :])

            # Linearized schedule: group g's remainder+relu defer into
            # group g+1 after its first chain (relaxing the SWDGE
            # wr4/xr4 deadline to ~19us), and group g's mm2+store defer
            # further into g+1 so the PE never waits on the relu chain
            # and the bias-adds clear the vector/scalar queues before
            # the next relu wave.  The last group's relus run
            # vector-only and its bias-add+store chain touches only
            # vector+sync, keeping the laggy scalar queue off the
            # critical tail.
            prev_ps1, prev_mm2 = None, None
            for g in range(NG):
                m = GROUP_SIZES[g]
                t0 = GROUP_START[g]
                last = g == NG - 1
                ps1s = []
                for j in range(m):
                    t = t0 + j
                    ps1 = ps1pool.tile([H, NB], f32, name="ps1")
                    for k in range(NK):
                        s = (t * NK + k) * NB
                        nc.tensor.matmul(
                            ps1[:],
                            lhsT=w_sb[:, k, :],
                            rhs=x_sb[:, s:s + NB],
                            start=(k == 0),
                            stop=False,
                        )
                    ps1s.append(ps1)
                    if j == 0 and prev_ps1 is not None:
                        prev_mm2 = (g - 1, rem_relu(g - 1, prev_ps1))
                        prev_ps1 = None
                    if j == min(2, m - 1) and prev_mm2 is not None:
                        mm2_store_burst(*prev_mm2,
                                        o_mode="scalar_full" if last
                                        else "split")
                        prev_mm2 = None
                prev_ps1 = ps1s
            hs = rem_relu(NG - 1, prev_ps1,
                          relu_fns={0: v_relu, 1: s_relu})
            mm2_store_burst(NG - 1, hs, o_mode="vector_final")

    nc.compile()
    return nc


def _get_nc():
    if "nc" not in _CACHE:
        _CACHE["nc"] = _build_nc()
    return _CACHE["nc"]


def _fold_weights(conv_w: np.ndarray, w1: np.ndarray) -> np.ndarray:
    """W_eff[784, 128]: h_pre = x @ W_eff  ==  conv(x) @ w1.T  (float64 accum)."""
    w1k = w1.reshape(H, 26, 26).transpose(1, 2, 0).astype(np.float64)  # [i,j,k]
    cw = conv_w.astype(np.float64)
    W = np.zeros((28, 28, H), np.float64)
    for di in range(3):
        for dj in range(3):
            W[di:di + 26, dj:dj + 26, :] += cw[di, dj] * w1k
    return W.reshape(D, H).astype(np.float32)


def make_in_maps(x, conv_w, w1, b1, w2, b2):
    x = np.asarray(x, np.float32)
    weff = _fold_weights(np.asarray(conv_w, np.float32),
                         np.asarray(w1, np.float32))
    wm = np.ascontiguousarray(
        weff[:DM].reshape(NK, KT, H).transpose(1, 0, 2)).astype(np.float16)
    wr4 = np.zeros((KT, H), np.float16)
    for j in range(4):
        wr4[32 * j:32 * j + DR] = weff[DM:].astype(np.float16)
    w2t = np.ascontiguousarray(np.asarray(w2, np.float32).T).astype(np.float16)
    biasd = np.zeros((KT, 2), np.float32)
    biasd[:, 0] = np.asarray(b1, np.float32)
    for j in range(4):
        biasd[32 * j:32 * j + OUT, 1] = np.asarray(b2, np.float32)
    in_maps = []
    for i in range(N_CORES):
        xq = x[i * B_SH:(i + 1) * B_SH].astype(ml_dtypes.float8_e3m4)
        xtp = xq[:, :DM].reshape(NT, NB, NK, KT).transpose(3, 0, 2, 1)
        xtp = np.ascontiguousarray(xtp).reshape(KT, NT * NK * NB)
        # remainder features into row-strip layout [32j+r, g, c]
        r16 = xq[:, DM:].reshape(NT, NB, DR)  # [t, c, r]
        xr4 = np.zeros((4, 32, NG, NB), ml_dtypes.float8_e3m4)
        for g in range(NG):
            for j in range(GROUP_SIZES[g]):
                t = GROUP_START[g] + j
                xr4[j, :DR, g, :] = r16[t].T
        in_maps.append({"xtp": xtp,
                        "xr4": np.ascontiguousarray(xr4.reshape(KT, NG, NB)),
                        "wm": wm, "wr4": wr4, "w2t": w2t, "biasd": biasd})
    return in_maps


def kernel(x, conv_w, w1, b1, w2, b2):
    nc = _get_nc()
    in_maps = make_in_maps(x, conv_w, w1, b1, w2, b2)
    res = run_bass_kernel_spmd(nc, in_maps, list(range(N_CORES)))
    # out4[32j+r, g, c] -> out[(GROUP_START[g]+j)*512+c, r]
    outs = []
    for i in range(N_CORES):
        o4 = res.results[i]["out4"].astype(np.float32)
        o4 = o4.reshape(4, 32, NG, NB)[:, :OUT]  # [j, r, g, c]
        core = np.empty((B_SH, OUT), np.float32)
        for g in range(NG):
            for j in range(GROUP_SIZES[g]):
                t = GROUP_START[g] + j
                core[t * NB:(t + 1) * NB] = o4[j, :, g, :].T
        outs.append(core)
    return np.ascontiguousarray(np.concatenate(outs, axis=0))  # [65536, 10]


# revision 26
# speedup vs baseline: 1.0092x; 1.0092x over previous
"""Trainium2 Bass kernel for DigitConvolutionalModel.

Math: the 3x3 valid conv is a linear map, so it folds into the first Linear
layer on the host (O(1) w.r.t. batch):  out = relu(x @ W_eff + b1) @ w2.T + b2
with W_eff[784, 128].  Distribution is pure data parallel: batch sharded
across 8 NeuronCores, weights replicated, each core computing [10, 8192].

dtypes: x ships as fp8 e3m4 (4 mantissa bits; subnormals handled exactly by
the PE's FP22 upconvert) against fp16 weights — this halves HBM traffic vs
fp16 and costs ~1.3e-2 max rel error on this data (gate: 2e-2).  fp32 PSUM
accumulation; h is emitted fp16 for the second matmul; logits are stored
fp16 (4.9e-4 relative — noise here) and upconverted on host.

PE-array packing: batch tiles process in GROUPS of [2,4,4,4,2].  Within a
group, tile j's K=16 remainder matmul runs in PE row-strip 32j via
tile_position=(32j, 0), and its M=10 second matmul runs in column strip
32j via tile_position=(0, 32j) — row/col-disjoint matmuls execute
concurrently, so a group's worth of remainder (or mm2) passes costs ~1
pass.  The 2-tile first group hands the DMA generator burst-slack right
where tile 2's delivery is tightest; the 2-tile last group halves the
end-of-kernel relu/mm2/store tail.

DMA plan: the two HWDGE rings (sync/scalar) share one descriptor
generator at ~20ns/descriptor; every SBUF-bound DMA costs one descriptor
per partition and lines cap at 8KB, so 2-tile x groups (6144B lines,
~305GB/s) are optimal — just above the PE's 296GB/s consumption.  The
generator serves commands in global trigger order (the rings' queues
interleave position by position), so the queue split below yields exactly
consumption order: wm, p01, p23, ..., p1415.  wm must lead a HWDGE ring:
the gpsimd SWDGE queue (which carries all small tensors and the
non-final output stores) only starts generating ~11.8us into the kernel.

The group epilogue pipeline: group g's mm2 burst is emitted after group
g+1's second mm1 chain, so the PE never waits on the relu chain and the
bias-adds clear the vector/scalar queues before the next relu wave.
Relus split vector/scalar per group; b2 is replicated per row-strip so
strip-sliced bias-adds work for any group size; each group stores its
strip block in one SWDGE DMA (final group: ring halves fired per
bias-add half).
"""

import numpy as np
import ml_dtypes

import concourse.bass as bass  # noqa: F401  (bass registers mybir lowerings)
import concourse.mybir as mybir
import concourse.tile as tile
from concourse import bacc
from concourse.bass_utils import run_bass_kernel_spmd

N_CORES = 8
B = 65536
B_SH = B // N_CORES  # 8192 rows per core
D = 784              # 28*28 input features
DM = 768             # features in the main 128-partition stream
DR = D - DM          # 16 remainder features
H = 128              # hidden
OUT = 10
KT = 128             # contraction tile = full partition dim
NK = DM // KT        # 6 main K-tiles
NB = 512             # batch columns per tile (= one fp32 PSUM bank)
NT = B_SH // NB      # 16 batch tiles

GROUP_SIZES = [2, 4, 4, 4, 2]
NG = len(GROUP_SIZES)
GROUP_START = [sum(GROUP_SIZES[:i]) for i in range(NG)]

N_WARM = 48  # bridge PE from engine-start (~7.5us) to wm+pair(0,1)
             # arrival (~12.5-13.4us) with no idle gap, so the HAM clock
             # never re-throttles (a >1us PE gap can halve the clock for
             # ~6us).  Warm passes use 256 cols — half the SBUF read
             # pressure of full passes, so they contend less with the
             # critical first x deliveries (~107-214ns each)

_CACHE = {}


def _build_nc():
    f32 = mybir.dt.float32
    f16 = mybir.dt.float16
    f8 = mybir.dt.float8e3
    nc = bacc.Bacc("TRN2", target_bir_lowering=False, debug=False,
                   num_devices=N_CORES)
    # x is flat per-partition so DMA commands can be exactly 8KB lines
    # (the descriptor max): byte s*NB..s*NB+NB of partition k is batch
    # tile s//NK, k-slice s%NK
    XBYTES = NT * NK * NB
    xtp = nc.dram_tensor("xtp", [KT, XBYTES], f8,
                         kind="ExternalInput").ap()
    # remainder features per row-strip: [32j+r, g, c] = feature 768+r of
    # group g's j-th tile (r<16; rows 16..31 of each strip are zero pad)
    xr4 = nc.dram_tensor("xr4", [KT, NG, NB], f8, kind="ExternalInput").ap()
    wm = nc.dram_tensor("wm", [KT, NK, H], f16, kind="ExternalInput").ap()
    # remainder weights replicated into each row-strip
    wr4 = nc.dram_tensor("wr4", [KT, H], f16, kind="ExternalInput").ap()
    w2t = nc.dram_tensor("w2t", [H, OUT], f16, kind="ExternalInput").ap()
    # biasd[:, 0] = b1; biasd[32j+r, 1] = b2[r] (r<10)
    biasd = nc.dram_tensor("biasd", [KT, 2], f32, kind="ExternalInput").ap()
    # out4[32j+r, g, c] = logit r of batch row (GROUP_START[g]+j)*512+c
    # (r<10; rows 10..31 of each strip, and strips >= group size, are
    # don't-care)
    out4 = nc.dram_tensor("out4", [KT, NG, NB], f16,
                          kind="ExternalOutput").ap()

    relu = mybir.ActivationFunctionType.Relu
    ident = mybir.ActivationFunctionType.Identity

    with tile.TileContext(nc) as tc:
        with (
            tc.tile_pool(name="wpool", bufs=1) as wpool,
            tc.tile_pool(name="xpool", bufs=1) as xpool,
            tc.tile_pool(name="hpool", bufs=8) as hpool,
            tc.tile_pool(name="opool", bufs=2) as opool,
            tc.tile_pool(name="ps1", bufs=6, space="PSUM") as ps1pool,
            tc.tile_pool(name="ps2", bufs=2, space="PSUM") as ps2pool,
        ):
            x_sb = xpool.tile([KT, XBYTES], f8)
            w_sb = wpool.tile([KT, NK, H], f16)
            w2_sb = wpool.tile([H, OUT], f16)
            bias_sb = wpool.tile([KT, 2], f32)
            wr_sb = wpool.tile([KT, H], f16)
            xr_sb = wpool.tile([KT, NG, NB], f8)

            # x streams in 2-tile 6KB-line commands (128 descriptors
            # each).  8KB commands are denser on paper but push more
            # bytes through the contended window right after the chain
            # starts (PE SBUF reads halve DMA write throughput), which
            # measured worse.  Queue split chosen so global trigger
            # order = consumption order: wm, p01, p23, ..., p1415.
            CH = 2 * NK * NB  # 6144 bytes/partition per pair
            nc.sync.dma_start(w_sb[:], wm[:])
            for ci in range(8):
                eng = (nc.scalar, nc.sync)[ci % 2]
                eng.dma_start(x_sb[:, ci * CH:(ci + 1) * CH],
                              xtp[:, ci * CH:(ci + 1) * CH])
            # SWDGE: remainder stream first (first rem burst needs
            # wr4/xr4 by ~15.5us), then bias (~16us), then w2t (~19us)
            nc.gpsimd.dma_start(wr_sb[:], wr4[:])
            nc.gpsimd.dma_start(xr_sb[:], xr4[:])
            nc.gpsimd.dma_start(bias_sb[:], biasd[:])
            nc.gpsimd.dma_start(w2_sb[:], w2t[:])

            warm_x = wpool.tile([KT, NB // 2], f16)
            nc.vector.memset(warm_x[:], 0.0)
            warm_ps = ps1pool.tile([H, NB], f32, tag="ps1")
            for _ in range(N_WARM):
                nc.tensor.matmul(warm_ps[:, 0:NB // 2], lhsT=warm_x[:, 0:H],
                                 rhs=warm_x[:], start=True, stop=True)

            def v_relu(h_sb, ps1):
                nc.vector.tensor_scalar(
                    h_sb[:], ps1[:], bias_sb[:, 0:1], 0.0,
                    mybir.AluOpType.add, mybir.AluOpType.max)

            def s_relu(h_sb, ps1):
                nc.scalar.activation(h_sb[:], ps1[:], relu,
                                     bias=bias_sb[:, 0:1])

            def rem_relu(g, ps1s, relu_fns=None):
                # remainder burst: row-tiled K=16 passes, one per strip,
                # then the relu wave split across DVE and Act so the
                # chain is <=2 deep per engine; emission order lets both
                # engines start immediately
                m = len(ps1s)
                for j in range(m):
                    nc.tensor.matmul(
                        ps1s[j][:],
                        lhsT=wr_sb[32 * j:32 * j + DR, :],
                        rhs=xr_sb[32 * j:32 * j + DR, g, :],
                        start=False, stop=True,
                        tile_position=(32 * j, 0),
                    )
                if relu_fns is None:
                    order = (0, 2, 1, 3) if m == 4 else (0, 1)
                    relu_fns = {j: (v_relu if j < m // 2 else s_relu)
                                for j in order}
                hs = [None] * m
                for j, fn in relu_fns.items():
                    h_sb = hpool.tile([H, NB], f16, name="h_sb")
                    fn(h_sb, ps1s[j])
                    hs[j] = h_sb
                return hs

            def mm2_store_burst(g, hs, o_mode="split"):
                # col-tiled mm2 passes into one shared PSUM bank; each
                # pass fires as its h lands (independent col strips)
                m = len(hs)
                ps2 = ps2pool.tile([KT, NB], f32, name="ps2")
                for j in range(m):
                    nc.tensor.matmul(
                        ps2[32 * j:32 * j + OUT, :],
                        lhsT=w2_sb[:], rhs=hs[j][:],
                        start=True, stop=True,
                        tile_position=(0, 32 * j),
                    )
                rows = 32 * m
                o_sb = opool.tile([KT, NB], f16, name="o_sb")
                if o_mode == "vector_final":
                    # final group: vector-only bias-add (the scalar
                    # queue shows ~1us of semaphore lag on trailing o
                    # ops), store halves on both now-idle HWDGE rings
                    nc.vector.tensor_scalar_add(
                        o_sb[0:rows, :], ps2[0:rows, :], bias_sb[0:rows, 1:2])
                    half = rows // 2
                    nc.sync.dma_start(out4[0:half, g, :], o_sb[0:half, :])
                    nc.scalar.dma_start(out4[half:rows, g, :],
                                        o_sb[half:rows, :])
                    return
                if o_mode == "scalar_full":
                    # penultimate group: keep the whole bias-add off the
                    # vector queue so the final relu wave owns it
                    nc.scalar.activation(
                        o_sb[0:rows, :], ps2[0:rows, :], ident,
                        bias=bias_sb[0:rows, 1:2])
                else:
                    half = rows // 2
                    nc.vector.tensor_scalar_add(
                        o_sb[0:half, :], ps2[0:half, :], bias_sb[0:half, 1:2])
                    nc.scalar.activation(
                        o_sb[half:rows, :], ps2[half:rows, :], ident,
                        bias=bias_sb[half:rows, 1:2])
                nc.gpsimd.dma_start(out4[0:rows, g, :], o_sb[0:rows, :])

            # Linearized schedule: group g's remainder+relu defer into
            # group g+1 after its first chain (relaxing the SWDGE
            # wr4/xr4 deadline to ~19us), and group g's mm2+store defer
            # further into g+1 so the PE never waits on the relu chain
            # and the bias-adds clear the vector/scalar queues before
            # the next relu wave.  The last group's relus run
            # vector-only and its bias-add+store chain touches only
            # vector+sync, keeping the laggy scalar queue off the
            # critical tail.
            prev_ps1, prev_mm2 = None, None
            for g in range(NG):
                m = GROUP_SIZES[g]
                t0 = GROUP_START[g]
                last = g == NG - 1
                ps1s = []
                for j in range(m):
                    t = t0 + j
                    ps1 = ps1pool.tile([H, NB], f32, name="ps1")
                    for k in range(NK):
                        s = (t * NK + k) * NB
                        nc.tensor.matmul(
                            ps1[:],
                            lhsT=w_sb[:, k, :],
                            rhs=x_sb[:, s:s + NB],
                            start=(k == 0),
                            stop=False,
                        )
                    ps1s.append(ps1)
                    if j == 0 and prev_ps1 is not None:
                        prev_mm2 = (g - 1, rem_relu(g - 1, prev_ps1))
                        prev_ps1 = None
                    if j == min(2, m - 1) and prev_mm2 is not None:
                        mm2_store_burst(*prev_mm2,
                                        o_mode="scalar_full" if last
                                        else "split")
                        prev_mm2 = None
                prev_ps1 = ps1s
            hs = rem_relu(NG - 1, prev_ps1,
                          relu_fns={0: v_relu, 1: s_relu})
            mm2_store_burst(NG - 1, hs, o_mode="vector_final")

    nc.compile()
    return nc


def _get_nc():
    if "nc" not in _CACHE:
        _CACHE["nc"] = _build_nc()
    return _CACHE["nc"]


def _fold_weights(conv_w: np.ndarray, w1: np.ndarray) -> np.ndarray:
    """W_eff[784, 128]: h_pre = x @ W_eff  ==  conv(x) @ w1.T  (float64 accum)."""
    w1k = w1.reshape(H, 26, 26).transpose(1, 2, 0).astype(np.float64)  # [i,j,k]
    cw = conv_w.astype(np.float64)
    W = np.zeros((28, 28, H), np.float64)
    for di in range(3):
        for dj in range(3):
            W[di:di + 26, dj:dj + 26, :] += cw[di, dj] * w1k
    return W.reshape(D, H).astype(np.float32)


def make_in_maps(x, conv_w, w1, b1, w2, b2):
    x = np.asarray(x, np.float32)
    weff = _fold_weights(np.asarray(conv_w, np.float32),
                         np.asarray(w1, np.float32))
    wm = np.ascontiguousarray(
        weff[:DM].reshape(NK, KT, H).transpose(1, 0, 2)).astype(np.float16)
    wr4 = np.zeros((KT, H), np.float16)
    for j in range(4):
        wr4[32 * j:32 * j + DR] = weff[DM:].astype(np.float16)
    w2t = np.ascontiguousarray(np.asarray(w2, np.float32).T).astype(np.float16)
    biasd = np.zeros((KT, 2), np.float32)
    biasd[:, 0] = np.asarray(b1, np.float32)
    for j in range(4):
        biasd[32 * j:32 * j + OUT, 1] = np.asarray(b2, np.float32)
    in_maps = []
    for i in range(N_CORES):
        xq = x[i * B_SH:(i + 1) * B_SH].astype(ml_dtypes.float8_e3m4)
        xtp = xq[:, :DM].reshape(NT, NB, NK, KT).transpose(3, 0, 2, 1)
        xtp = np.ascontiguousarray(xtp).reshape(KT, NT * NK * NB)
        # remainder features into row-strip layout [32j+r, g, c]
        r16 = xq[:, DM:].reshape(NT, NB, DR)  # [t, c, r]
        xr4 = np.zeros((4, 32, NG, NB), ml_dtypes.float8_e3m4)
        for g in range(NG):
            for j in range(GROUP_SIZES[g]):
                t = GROUP_START[g] + j
                xr4[j, :DR, g, :] = r16[t].T
        in_maps.append({"xtp": xtp,
                        "xr4": np.ascontiguousarray(xr4.reshape(KT, NG, NB)),
                        "wm": wm, "wr4": wr4, "w2t": w2t, "biasd": biasd})
    return in_maps


def kernel(x, conv_w, w1, b1, w2, b2):
    nc = _get_nc()
    in_maps = make_in_maps(x, conv_w, w1, b1, w2, b2)
    res = run_bass_kernel_spmd(nc, in_maps, list(range(N_CORES)))
    # out4[32j+r, g, c] -> out[(GROUP_START[g]+j)*512+c, r]
    outs = []
    for i in range(N_CORES):
        o4 = res.results[i]["out4"].astype(np.float32)
        o4 = o4.reshape(4, 32, NG, NB)[:, :OUT]  # [j, r, g, c]
        core = np.empty((B_SH, OUT), np.float32)
        for g in range(NG):
            for j in range(GROUP_SIZES[g]):
                t = GROUP_START[g] + j
                core[t * NB:(t + 1) * NB] = o4[j, :, g, :].T
        outs.append(core)
    return np.ascontiguousarray(np.concatenate(outs, axis=0))  # [65536, 10]
